# revision 3
# baseline (speedup 1.0000x reference)
"""Trainium2 Bass kernel for the FlowNet-style CorrelationLayer.

Problem: x1, x2 [B=4, C=128, H=64, W=64] f32; out [B, 41*41, H, W] where
  out[b, (di*41+dj), h, w] = mean_c x1[b,c,h,w] * x2pad[b,c,h+di,w+dj]
(x2 zero-padded by 20 in H and W; di,dj in [0,41)).

Sharding: 8 cores = (batch b, H-half q). Each core handles 32 rows of one
batch image; x2 slab includes the +/-20 halo rows (host-padded, so the
kernel has no boundary logic).

Per-core algorithm:
  For each pair of rows (h0, h0+1): stationary = x1[:, rows h0,h0+1] as a
  [C=128, 128] matrix. For the 42 relevant padded x2 rows s (in groups of
  3 -> N=312 moving), matmul gives M[(hs,w), (s,w')] = sum_c x1*x2 -- the
  correlation band lives on the diagonals M[(hs,w), (hs+di, w+dj)].
  Band extraction is a shear, only expressible as a flat-address DMA
  access pattern; PE-transpose then flips [w-major] -> [channel-major]
  tiles which DMA out with 2KB contiguous runs.

Matmuls run as float32r (TF32-like, 1 cyc/row); transposes in exact f32.
"""

import os
import numpy as np

B, C, H, W = 4, 128, 64, 64
MAXD = 20
D = 2 * MAXD + 1            # 41
NCH = D * D                 # 1681
HH = H // 2                 # 32 rows per core
WP = W + 2 * MAXD           # 104 padded width
SROWS = HH + 2 * MAXD       # 72 slab rows per core
PAIR_ROWS = 42              # x2 rows touched by one h-pair
SG = 3                      # s-rows per matmul (N = 312)
NSG = PAIR_ROWS // SG       # 14 matmuls per h-pair
NPAIR = HH // 2             # 16 h-pairs
NCHUNK = (NCH + 127) // 128  # 14 channel chunks (13x128 + 17)
FSM = PAIR_ROWS * WP        # 4368 = m_sb free size
FLUSH = 4                   # h-pairs per output flush (8 h rows)

# band extraction mode: "sbuf16" = diagonal SBUF->SBUF DMAs in <=16
# partition chunks; "dram" = round-trip M through DRAM, diagonal on the
# (flat, safe) DRAM side.
DIAG_MODE = os.environ.get("CORR_DIAG_MODE", "sbuf16")
DIAG_CHUNK = int(os.environ.get("CORR_DIAG_CHUNK", "16"))

_CACHE = {}


def _patch_drain(tile_mod, ScopedClock):
    if getattr(tile_mod.TileContext, "_corr_drain_patched", False):
        return

    def _drain_and_barrier(self, tick_clock, wait_clock):
        nc = self.nc
        lead = nc.sync.nop(nofuse=True)
        wait_clock.add_sem_waits(
            lead.ins, ScopedClock({None: tick_clock.global_clock})
        )
        waits = list(lead.ins.sync_info.on_wait or []) if lead.ins.sync_info else []
        if len(waits) > 1:
            lead.ins.sync_info.on_wait = waits[:1]
            for w in waits[1:]:
                extra = nc.sync.nop(nofuse=True)
                if extra.ins.sync_info is None:
                    wait_clock.add_sem_waits(
                        extra.ins, ScopedClock({None: tick_clock.global_clock})
                    )
                extra.ins.sync_info.on_wait = [w]
        # the nops above carry every outstanding wait in-order on SP, so the
        # drain itself needs none (this walrus build caps waits per CTRL).
        nc.sync.drain()
        nc.all_engine_barrier()
        assert self.sems is not None
        popped = nc._tile_sem_poison_stack.pop()
        assert popped is self._sem_poison
        nc.clear_and_free_semaphores(list(self.sems.allocated().values()))
        nc.all_engine_barrier()

    tile_mod.TileContext._drain_and_barrier = _drain_and_barrier
    tile_mod.TileContext._corr_drain_patched = True


def _split_excess_waits(nc, mybir, limit=1):
    """This walrus build rejects instructions carrying more than ~2 sem
    waits ("Too many sync wait commands"). Hoist all but `limit` waits of
    every instruction onto same-engine NoOps inserted right before it —
    per-engine program order makes that equivalent."""
    import bass_rust

    n = 0
    for f in nc.m.functions:
        for bb in f.blocks:
            new = []
            changed = False
            for inst in bb.instructions:
                si = getattr(inst, "sync_info", None)
                waits = list(si.on_wait) if si is not None and si.on_wait else []
                if len(waits) > limit:
                    for w in waits[: len(waits) - limit]:
                        nop = mybir.InstNoOp(name=f"I-wx{n}", ins=[], outs=[])
                        n += 1
                        nop.engine = inst.engine
                        nop.sync_info = bass_rust.SyncInfo(on_wait=[w], on_update=[])
                        new.append(nop)
                    si.on_wait = waits[len(waits) - limit :]
                    changed = True
                new.append(inst)
            if changed:
                try:
                    bb.instructions = new
                except Exception:
                    bb.instructions[:] = new
    return n


def _build():
    import concourse.bass as bass
    import concourse.tile as tile_mod
    from concourse import mybir
    from concourse.masks import make_identity
    from concourse.vector_clock import ScopedClock
    from contextlib import ExitStack

    _patch_drain(tile_mod, ScopedClock)

    F32 = mybir.dt.float32
    F32R = mybir.dt.float32r

    nc = bass.Bass("TRN2", target_bir_lowering=False, debug=False)
    x1_d = nc.dram_tensor("x1", [C, HH * W], F32R, kind="ExternalInput")
    x2_d = nc.dram_tensor("x2", [C, SROWS * WP], F32R, kind="ExternalInput")
    out_d = nc.dram_tensor("out", [NCH, HH * W], F32, kind="ExternalOutput")

    copy_i = [0]

    with tile_mod.TileContext(nc) as tc, ExitStack() as ctx:
        singles = ctx.enter_context(tc.tile_pool(name="singles", bufs=1))
        mpool = ctx.enter_context(tc.tile_pool(name="mpool", bufs=2))
        bandp = ctx.enter_context(tc.tile_pool(name="bandp", bufs=2))
        outp = ctx.enter_context(tc.tile_pool(name="outp", bufs=2))
        psum_mm = ctx.enter_context(
            tc.tile_pool(name="psum_mm", bufs=3, space="PSUM")
        )
        psum_t = ctx.enter_context(
            tc.tile_pool(name="psum_t", bufs=3, space="PSUM")
        )
        dram = None
        if DIAG_MODE == "dram":
            dram = ctx.enter_context(tc.tile_pool(name="dram", bufs=2, space="DRAM"))

        ident = singles.tile([128, 128], F32)
        make_identity(nc, ident)
        x1_sb = singles.tile([C, HH * W], F32R)
        nc.sync.dma_start(x1_sb[:], x1_d.ap())
        x2_sb = singles.tile([C, SROWS * WP], F32R)
        nc.sync.dma_start(x2_sb[:], x2_d.ap())

        def copy_alt(out_ap, in_ap):
            if copy_i[0] % 2 == 0:
                nc.vector.tensor_copy(out=out_ap, in_=in_ap)
            else:
                nc.scalar.copy(out=out_ap, in_=in_ap)
            copy_i[0] += 1

        for fg in range(NPAIR // FLUSH):
            out_ts = [
                outp.tile([128, FLUSH * 128], F32, name=f"ot{fg}_{t}", tag=f"ot{t}")
                for t in range(NCHUNK)
            ]
            for hpi in range(FLUSH):
                hp = fg * FLUSH + hpi
                h0 = 2 * hp
                m_sb = mpool.tile([128, FSM], F32, name=f"m{hp}", tag="m")
                for g in range(NSG):
                    s0 = h0 + SG * g
                    ps = psum_mm.tile([128, SG * WP], F32, name=f"ps{hp}_{g}", tag="mm")
                    nc.tensor.matmul(
                        ps[:],
                        x1_sb[:, h0 * W : h0 * W + 128],
                        x2_sb[:, s0 * WP : (s0 + SG) * WP],
                        start=True,
                        stop=True,
                    )
                    copy_alt(m_sb[:, SG * g * WP : SG * (g + 1) * WP], ps[:])

                band = bandp.tile([128, NCH], F32, name=f"b{hp}", tag="band")
                if DIAG_MODE == "dram":
                    md = dram.tile([128, FSM], F32, name=f"md{hp}", tag="md")
                    nc.sync.dma_start(md[:], m_sb[:])
                    for hs in (0, 1):
                        src = bass.AP(
                            tensor=md.tensor,
                            offset=md.offset + hs * (64 * FSM + WP),
                            ap=[[FSM + 1, 64], [WP, D], [1, D]],
                        )
                        nc.sync.dma_start(band[hs * 64 : (hs + 1) * 64, :], src)
                else:
                    ck = DIAG_CHUNK
                    for hs in (0, 1):
                        for wc in range(0, 64, ck):
                            src = bass.AP(
                                tensor=m_sb.tensor,
                                offset=m_sb.offset
                                + hs * (64 * FSM + WP)
                                + wc * (FSM + 1),
                                ap=[[FSM + 1, ck], [WP, D], [1, D]],
                            )
                            nc.gpsimd.dma_start(
                                band[hs * 64 + wc : hs * 64 + wc + ck, :], src
                            )

                for t in range(NCHUNK):
                    cw = 128 if t < NCHUNK - 1 else NCH - 128 * (NCHUNK - 1)
                    pt = psum_t.tile([128, 128], F32, name=f"pt{hp}_{t}", tag="tp")
                    nc.tensor.transpose(
                        pt[:cw, :], band[:, t * 128 : t * 128 + cw], ident[:]
                    )
                    copy_alt(
                        out_ts[t][:cw, hpi * 128 : (hpi + 1) * 128], pt[:cw, :]
                    )
            for t in range(NCHUNK):
                cw = 128 if t < NCHUNK - 1 else NCH - 128 * (NCHUNK - 1)
                nc.sync.dma_start(
                    out_d.ap()[
                        t * 128 : t * 128 + cw,
                        fg * FLUSH * 128 : (fg + 1) * FLUSH * 128,
                    ],
                    out_ts[t][:cw, :],
                )
    _split_excess_waits(nc, mybir, limit=1)
    return nc


def kernel(x1, x2):
    from concourse.bass_utils import run_bass_kernel_spmd

    x1 = np.ascontiguousarray(np.asarray(x1, dtype=np.float32))
    x2 = np.ascontiguousarray(np.asarray(x2, dtype=np.float32))

    x1s = x1 * np.float32(1.0 / C)
    x2p = np.pad(x2, ((0, 0), (0, 0), (MAXD, MAXD), (MAXD, MAXD)))

    in_maps = []
    for core in range(8):
        b, q = core // 2, core % 2
        a1 = np.ascontiguousarray(
            x1s[b, :, q * HH : (q + 1) * HH, :].reshape(C, HH * W)
        )
        a2 = np.ascontiguousarray(
            x2p[b, :, q * HH : q * HH + SROWS, :].reshape(C, SROWS * WP)
        )
        in_maps.append({"x1": a1, "x2": a2})

    if "nc" not in _CACHE:
        _CACHE["nc"] = _build()
    nc = _CACHE["nc"]

    res = run_bass_kernel_spmd(nc, in_maps, core_ids=list(range(8)))
    globals()["LAST_RESULT"] = res

    out = np.empty((B, NCH, H, W), dtype=np.float32)
    for core in range(8):
        b, q = core // 2, core % 2
        out[b, :, q * HH : (q + 1) * HH, :] = res.results[core]["out"].reshape(
            NCH, HH, W
        )
    return out
